# revision 12
# baseline (speedup 1.0000x reference)
"""Trainium2 Bass kernel for nn_RNN2Classifier (Elman RNN H=2, T=4 + linear head).

Math (all weights are compile-time constants):
  h_t = tanh(W_ih x_t + b_ih + W_hh h_{t-1} + b_hh),  h_0 = 0
  out = cls_w . h_4 + cls_b

v3 scheme: the whole recurrence runs on the PE as scaled-identity matmuls
accumulating in PSUM.  Batch lives on partitions 0-126 x free columns; SBUF
partition 127 of every input tile holds the constant 1.0, and the u-term
stationary carries the bias on its row 127, so each z_t is produced entirely
inside one PSUM accumulation group:

  z[t,h] = Su[h].T@u_t + Sv[h].T@v_t + Sp0[h].T@h0_{t-1} + Sp1[h].T@h1_{t-1}
  Su[h] = diag(WI[h][0]) with row127 = BI[h]+BH[h]     (bias via ones-row)
  Sv[h] = diag(WI[h][1]);  Sp*[h] = diag(WH[h][*]);  Sc[h] = diag(CW[h])

ACT then applies one packed tanh per t over both h (z0 in psum cols [0:G],
z1 in [512:512+G] so each matmul output stays inside one PSUM bank).

Sharding: pure data parallel, batch split 8 ways (500k rows/core).  The host
pre-packs each core's slice to [128, 8*G_TOTAL] with u/v de-interleaved per
chunk so every matmul rhs is a contiguous [128, G] slice, and the DMA uses
all 128 partitions (~310 GB/s vs ~120 GB/s at 125 partitions).
"""

import sys

import numpy as np

sys.path.insert(0, "/opt/trn_rl_repo")

N_CORES = 8
B_TOTAL = 4_000_000
B_CORE = B_TOTAL // N_CORES  # 500_000
P = 127                      # batch partitions; partition 127 = ones row
G = 493                      # columns per chunk (<= 512 = one PSUM bank of f32)
N_CHUNKS = 8
N_PAIRS = N_CHUNKS // 2
G_TOTAL = G * N_CHUNKS       # 3944
B_PAD = P * G_TOTAL          # 500_888 (>= B_CORE)
CHUNK_COLS = 8 * G           # 3944 = (u:4G, v:4G) per chunk
COLS = N_CHUNKS * CHUNK_COLS  # 31552

WI = [[0.3519, -0.6514], [0.3238, 0.5568]]
BI = [0.2198, 0.4712]
WH = [[0.4279, 0.6832], [-0.4114, 0.5715]]
BH = [-0.409, -0.1299]
CW = [-0.2732, -0.1587]
CB = 0.5806

_cached_nc = None


def build_program():
    import concourse.bass as bass  # noqa: F401
    import concourse.tile as tile
    from concourse import bacc, mybir

    f32 = mybir.dt.float32
    Tanh = mybir.ActivationFunctionType.Tanh
    Copy = mybir.ActivationFunctionType.Copy

    nc = bacc.Bacc(None, target_bir_lowering=False)

    # Tanh bias is lowered via the const-AP database; register 0.0.
    cb = nc.alloc_sbuf_tensor("const-bias-zero", [128, 1], f32)
    nc.gpsimd.memset(cb.ap(), 0.0)
    nc.const_aps.aps[(f32, 0.0)] = cb.ap()

    x_d = nc.dram_tensor("X", [128, COLS], f32, kind="ExternalInput")
    o_d = nc.dram_tensor("out", [128, G_TOTAL], f32, kind="ExternalOutput")
    xv = x_d[:]
    ov = o_d[:]

    HOFF = 512  # h1 column offset inside [128, 1024] z/h tiles (bank aligned)

    def packed(tile_ap):
        # [128, 1024] -> [128, 2, G]: both h blocks, skipping unwritten gaps
        return tile_ap.rearrange("p (b c) -> p b c", b=2)[:, :, 0:G]

    with tile.TileContext(nc) as tc:
        with (
            tc.tile_pool(name="io", bufs=3) as io_pool,
            tc.tile_pool(name="hid", bufs=1) as hpool,
            tc.tile_pool(name="res", bufs=1) as opool,
            tc.tile_pool(name="stat", bufs=1) as spool,
            tc.tile_pool(name="ps", bufs=1, space="PSUM") as ps,
        ):

            def diag_stationary(name, dval, row127=None):
                ap = spool.tile([128, 128], f32, tag=name)
                nc.gpsimd.memset(ap, 0.0)
                nc.gpsimd.affine_select(
                    out=ap,
                    in_=ap,
                    compare_op=mybir.AluOpType.not_equal,
                    fill=dval,
                    base=0,
                    pattern=[[-1, 128]],
                    channel_multiplier=1,
                )
                if row127 is not None:
                    # bias enters via rhs row 127 == 1.0; only out partition
                    # 127 (garbage lane) sees the clobbered diagonal element.
                    nc.gpsimd.affine_select(
                        out=ap,
                        in_=ap,
                        compare_op=mybir.AluOpType.not_equal,
                        fill=row127,
                        base=-127,
                        pattern=[[0, 128]],
                        channel_multiplier=1,
                    )
                return ap

            su = [
                diag_stationary(f"su{h}", WI[h][0], BI[h] + BH[h])
                for h in range(2)
            ]
            sv = [diag_stationary(f"sv{h}", WI[h][1]) for h in range(2)]
            sp = [
                [diag_stationary(f"sp{h}{k}", WH[h][k]) for k in range(2)]
                for h in range(2)
            ]
            sc = [diag_stationary(f"sc{h}", CW[h]) for h in range(2)]

            out_acc = opool.tile([128, G_TOTAL], f32, tag="out_acc")
            for pair in range(N_PAIRS):
                js = (2 * pair, 2 * pair + 1)
                xts = []
                for jj, j in enumerate(js):
                    xt = io_pool.tile([128, CHUNK_COLS], f32, tag=f"xt{jj}")
                    nc.sync.dma_start(
                        out=xt, in_=xv[:, j * CHUNK_COLS : (j + 1) * CHUNK_COLS]
                    )
                    xts.append(xt)

                def ut(jj, t):
                    return xts[jj][:, t * G : (t + 1) * G]

                def vt(jj, t):
                    return xts[jj][:, 4 * G + t * G : 4 * G + (t + 1) * G]

                zs = {}
                hs = {}
                # t = 0: z = Su@u + Sv@v  (bias comes from Su row 127)
                for jj in range(2):
                    zp = ps.tile([128, 1024], f32, tag=f"zp{jj}0")
                    for h in range(2):
                        o = zp[:, HOFF * h : HOFF * h + G]
                        nc.tensor.matmul(o, su[h], ut(jj, 0), start=True, stop=False)
                        nc.tensor.matmul(o, sv[h], vt(jj, 0), start=False, stop=True)
                    zs[jj] = zp
                for jj in range(2):
                    ht = hpool.tile([128, 1024], f32, tag=f"h{jj}0")
                    nc.scalar.activation(
                        out=packed(ht), in_=packed(zs[jj]),
                        func=Tanh, bias=0.0, scale=1.0,
                    )
                    hs[(jj, 0)] = ht

                for t in (1, 2, 3):
                    for jj in range(2):
                        zp = ps.tile([128, 1024], f32, tag=f"zp{jj}{t % 2}")
                        hp = hs[(jj, t - 1)]
                        for h in range(2):
                            o = zp[:, HOFF * h : HOFF * h + G]
                            nc.tensor.matmul(o, su[h], ut(jj, t), start=True, stop=False)
                            nc.tensor.matmul(o, sv[h], vt(jj, t), start=False, stop=False)
                            nc.tensor.matmul(o, sp[h][0], hp[:, 0:G], start=False, stop=False)
                            nc.tensor.matmul(
                                o, sp[h][1], hp[:, HOFF : HOFF + G], start=False, stop=True
                            )
                        zs[jj] = zp
                    for jj in range(2):
                        ht = hpool.tile([128, 1024], f32, tag=f"h{jj}{t}")
                        nc.scalar.activation(
                            out=packed(ht), in_=packed(zs[jj]),
                            func=Tanh, bias=0.0, scale=1.0,
                        )
                        hs[(jj, t)] = ht

                # classifier: q = Sc0@h0_3 + Sc1@h1_3 ; out = q + CB
                # reuses tag zp{jj}1 -- its WAR wait (tanh t3 read) coincides
                # with the RAW need (h3 ready), so no extra PE stall.
                for jj in range(2):
                    zp = ps.tile([128, 1024], f32, tag=f"zp{jj}1")
                    h3 = hs[(jj, 3)]
                    nc.tensor.matmul(zp[:, 0:G], sc[0], h3[:, 0:G], start=True, stop=False)
                    nc.tensor.matmul(
                        zp[:, 0:G], sc[1], h3[:, HOFF : HOFF + G], start=False, stop=True
                    )
                    zs[jj] = zp
                for jj, j in enumerate(js):
                    nc.scalar.activation(
                        out=out_acc[:, j * G : (j + 1) * G],
                        in_=zs[jj][:, 0:G],
                        func=Copy,
                        bias=CB,
                        scale=1.0,
                    )

                nc.gpsimd.dma_start(
                    out=ov[:, pair * 2 * G : (pair + 1) * 2 * G],
                    in_=out_acc[:, pair * 2 * G : (pair + 1) * 2 * G],
                )

    nc.compile()
    return nc


def _get_nc():
    global _cached_nc
    if _cached_nc is None:
        _cached_nc = build_program()
    return _cached_nc


def _pack_core(xc: np.ndarray) -> np.ndarray:
    """[B_CORE, 4, 2] -> [128, COLS] with per-chunk u/v de-interleave."""
    pad = np.zeros((B_PAD, 8), np.float32)
    pad[:B_CORE] = xc.reshape(B_CORE, 8)
    v = pad.reshape(P, N_CHUNKS, G, 4, 2).transpose(0, 1, 4, 3, 2)
    buf = np.empty((128, COLS), np.float32)
    buf[:P] = v.reshape(P, COLS)
    buf[P] = 1.0
    return buf


def run_sharded(X: np.ndarray, trace: bool = False):
    """Run the SPMD kernel on 8 cores. Returns (out_full, BassKernelResults)."""
    from concourse import bass_utils

    nc = _get_nc()
    X = np.ascontiguousarray(np.asarray(X, dtype=np.float32))
    assert X.shape == (B_TOTAL, 4, 2), X.shape
    in_maps = [
        {"X": _pack_core(X[i * B_CORE : (i + 1) * B_CORE])}
        for i in range(N_CORES)
    ]
    res = bass_utils.run_bass_kernel_spmd(
        nc, in_maps, core_ids=list(range(N_CORES)), trace=trace
    )
    out = np.concatenate(
        [
            res.results[i]["out"][:P].reshape(-1)[:B_CORE]
            for i in range(N_CORES)
        ],
        axis=0,
    ).reshape(B_TOTAL, 1)
    return out, res


def kernel(**inputs: np.ndarray) -> np.ndarray:
    out, _ = run_sharded(inputs["X"])
    return out.astype(np.float32)


# revision 13
# speedup vs baseline: 1.8594x; 1.8594x over previous
"""Trainium2 Bass kernel for nn_RNN2Classifier (Elman RNN H=2, T=4 + linear head).

Math (all weights are compile-time constants):
  h_t = tanh(W_ih x_t + b_ih + W_hh h_{t-1} + b_hh),  h_0 = 0
  out = cls_w . h_4 + cls_b

Factoring (one scalar per op, matching scalar_tensor_tensor):
  s_th  = (u_t * wi[h,0]/wi[h,1]) + v_t                      [STT]
  z_th  = (hp0 * wh[h,0]/wi[h,1]) + s_th ; z += hp1 * wh[h,1]/wi[h,1]
  h_th  = tanh(wi[h,1] * z + (bi[h]+bh[h]))                  [ACT, scale+bias free]
  q     = (h_30 * cw0/cw1) + h_31 ; out = Copy(cw1 * q + cb) [STT + ACT]

Sharding: pure data parallel, batch split 8 ways (500k rows/core).  The host
pre-packs each core's slice to [128, 8*G_TOTAL] with u (=x[...,0]) and
v (=x[...,1]) de-interleaved per chunk so every DVE operand is a contiguous
slice, and the DMA uses all 128 partitions (~310 GB/s vs ~120 GB/s at 125).
"""

import sys

import numpy as np

sys.path.insert(0, "/opt/trn_rl_repo")

N_CORES = 8
B_TOTAL = 4_000_000
B_CORE = B_TOTAL // N_CORES  # 500_000
P = 127                      # batch partitions; partition 127 is padding
G = 985                      # columns per chunk
N_CHUNKS = 4
G_TOTAL = G * N_CHUNKS       # 3940
B_PAD = P * G_TOTAL          # 500_380 (>= B_CORE)
CHUNK_COLS = 8 * G           # 7880 = (u:4G, v:4G) per chunk
COLS = N_CHUNKS * CHUNK_COLS  # 31520

WI = [[0.3519, -0.6514], [0.3238, 0.5568]]
BI = [0.2198, 0.4712]
WH = [[0.4279, 0.6832], [-0.4114, 0.5715]]
BH = [-0.409, -0.1299]
CW = [-0.2732, -0.1587]
CB = 0.5806

_cached_nc = None


def build_program():
    import concourse.bass as bass  # noqa: F401
    import concourse.tile as tile
    from concourse import bacc, mybir

    f32 = mybir.dt.float32
    mult = mybir.AluOpType.mult
    add = mybir.AluOpType.add
    Tanh = mybir.ActivationFunctionType.Tanh
    Copy = mybir.ActivationFunctionType.Copy

    r = [WI[0][0] / WI[0][1], WI[1][0] / WI[1][1]]
    a = [
        [WH[0][0] / WI[0][1], WH[0][1] / WI[0][1]],
        [WH[1][0] / WI[1][1], WH[1][1] / WI[1][1]],
    ]
    act_scale = [WI[0][1], WI[1][1]]
    act_bias = [BI[0] + BH[0], BI[1] + BH[1]]
    ccoef = CW[0] / CW[1]

    nc = bacc.Bacc(None, target_bir_lowering=False)

    # Tanh bias is lowered via the const-AP database; register our values.
    for val in act_bias:
        t = nc.alloc_sbuf_tensor(f"const-bias-{val}", [128, 1], f32)
        nc.gpsimd.memset(t.ap(), val)
        nc.const_aps.aps[(f32, val)] = t.ap()
    nc.all_engine_barrier()

    x_d = nc.dram_tensor("X", [128, COLS], f32, kind="ExternalInput")
    o_d = nc.dram_tensor("out", [128, G_TOTAL], f32, kind="ExternalOutput")
    xv = x_d[:]
    ov = o_d[:]

    with tile.TileContext(nc) as tc:
        with (
            tc.tile_pool(name="io", bufs=2) as io_pool,
            tc.tile_pool(name="work", bufs=2) as work,
            tc.tile_pool(name="persist", bufs=1) as persist,
        ):
            out_acc = persist.tile([128, G_TOTAL], f32, tag="out_acc")
            for j in range(N_CHUNKS):
                xt = io_pool.tile([128, CHUNK_COLS], f32, tag="xt")
                nc.sync.dma_start(
                    out=xt, in_=xv[:, j * CHUNK_COLS : (j + 1) * CHUNK_COLS]
                )
                u_all = xt[:, 0 : 4 * G]
                v_all = xt[:, 4 * G : 8 * G]

                # input projections for all 4 timesteps, one op per h
                s = []
                for h in range(2):
                    sh = work.tile([128, 4 * G], f32, tag=f"s{h}", bufs=1)
                    nc.vector.scalar_tensor_tensor(
                        out=sh, in0=u_all, scalar=r[h], in1=v_all,
                        op0=mult, op1=add,
                    )
                    s.append(sh)

                # t = 0
                hcur = []
                for h in range(2):
                    ht = work.tile([128, G], f32, tag=f"h{h}0", bufs=1)
                    nc.scalar.activation(
                        out=ht,
                        in_=s[h][:, 0:G],
                        func=Tanh,
                        bias=act_bias[h],
                        scale=act_scale[h],
                    )
                    hcur.append(ht)

                # t = 1..3: per h two DVE STT + one ACT tanh
                for t in range(1, 4):
                    hprev = hcur
                    hcur = []
                    for h in range(2):
                        tmp = work.tile([128, G], f32, tag=f"tmp{h}")
                        nc.vector.scalar_tensor_tensor(
                            out=tmp,
                            in0=hprev[0],
                            scalar=a[h][0],
                            in1=s[h][:, t * G : (t + 1) * G],
                            op0=mult,
                            op1=add,
                        )
                        z = work.tile([128, G], f32, tag=f"z{h}")
                        nc.vector.scalar_tensor_tensor(
                            out=z,
                            in0=hprev[1],
                            scalar=a[h][1],
                            in1=tmp,
                            op0=mult,
                            op1=add,
                        )
                        ht = work.tile([128, G], f32, tag=f"h{h}{t % 2}", bufs=1)
                        nc.scalar.activation(
                            out=ht,
                            in_=z,
                            func=Tanh,
                            bias=act_bias[h],
                            scale=act_scale[h],
                        )
                        hcur.append(ht)

                # classifier
                q = work.tile([128, G], f32, tag="q")
                nc.vector.scalar_tensor_tensor(
                    out=q,
                    in0=hcur[0],
                    scalar=ccoef,
                    in1=hcur[1],
                    op0=mult,
                    op1=add,
                )
                nc.scalar.activation(
                    out=out_acc[:, j * G : (j + 1) * G],
                    in_=q,
                    func=Copy,
                    bias=CB,
                    scale=CW[1],
                )
                nc.gpsimd.dma_start(
                    out=ov[:, j * G : (j + 1) * G],
                    in_=out_acc[:, j * G : (j + 1) * G],
                )

    nc.compile()
    return nc


def _get_nc():
    global _cached_nc
    if _cached_nc is None:
        _cached_nc = build_program()
    return _cached_nc


def _pack_core(xc: np.ndarray) -> np.ndarray:
    """[B_CORE, 4, 2] -> [128, COLS] with per-chunk u/v de-interleave."""
    pad = np.zeros((B_PAD, 8), np.float32)
    pad[:B_CORE] = xc.reshape(B_CORE, 8)
    v = pad.reshape(P, N_CHUNKS, G, 4, 2).transpose(0, 1, 4, 3, 2)
    buf = np.zeros((128, COLS), np.float32)
    buf[:P] = v.reshape(P, COLS)
    return buf


def run_sharded(X: np.ndarray, trace: bool = False):
    """Run the SPMD kernel on 8 cores. Returns (out_full, BassKernelResults)."""
    from concourse import bass_utils

    nc = _get_nc()
    X = np.ascontiguousarray(np.asarray(X, dtype=np.float32))
    assert X.shape == (B_TOTAL, 4, 2), X.shape
    in_maps = [
        {"X": _pack_core(X[i * B_CORE : (i + 1) * B_CORE])}
        for i in range(N_CORES)
    ]
    res = bass_utils.run_bass_kernel_spmd(
        nc, in_maps, core_ids=list(range(N_CORES)), trace=trace
    )
    out = np.concatenate(
        [
            res.results[i]["out"][:P].reshape(-1)[:B_CORE]
            for i in range(N_CORES)
        ],
        axis=0,
    ).reshape(B_TOTAL, 1)
    return out, res


def kernel(**inputs: np.ndarray) -> np.ndarray:
    out, _ = run_sharded(inputs["X"])
    return out.astype(np.float32)


# revision 14
# speedup vs baseline: 2.1954x; 1.1807x over previous
"""Trainium2 Bass kernel for nn_RNN2Classifier (Elman RNN H=2, T=4 + linear head).

Math (all weights are compile-time constants):
  h_t = tanh(W_ih x_t + b_ih + W_hh h_{t-1} + b_hh),  h_0 = 0
  out = cls_w . h_4 + cls_b

Hybrid v5: the batch is split between two compute pipelines that run
concurrently on different engines (sharing only ACT for tanh):

DVE slice (127 partitions x G_TOTAL_D cols), per (t,h) two STT + one tanh:
  s_th  = (u_t * wi[h,0]/wi[h,1]) + v_t                      [STT]
  z_th  = (hp0 * wh[h,0]/wi[h,1]) + s_th ; z += hp1 * wh[h,1]/wi[h,1]
  h_th  = tanh(wi[h,1] * z + (bi[h]+bh[h]))                  [ACT]

PE slice (block-diagonal matmuls, 64 batch groups x 2 features per
partition, 512 batch elems per column block, PSUM accumulation):
  z_t[2g+h, c] = sum_k WI[h][k] x_k + sum_k WH[h][k] hp_k    [2 MMs]
  h_t = tanh(z_t + bias_vec)       bias_vec[2g+h] = bi[h]+bh[h]  [ACT, AP bias]
  q[g, c] = sum_k CW[k] h3[2g+k]                             [1 MM]
Block-diag weights / bias vector are host-built and DMA'd once ("CT").

Sharding: pure data parallel, batch split 8 ways (500k rows/core).  Both
input packs use all 128 DMA partitions with contiguous per-operand slices.
"""

import sys

import numpy as np

sys.path.insert(0, "/opt/trn_rl_repo")

N_CORES = 8
B_TOTAL = 4_000_000
B_CORE = B_TOTAL // N_CORES  # 500_000

# ---- PE slice geometry ----
NBLK = 6                     # 512-column blocks (must be even: processed in pairs)
BLK_ELEMS = 64 * 512         # 64 groups x 512 cols
B_PE = NBLK * BLK_ELEMS      # 196_608
PE_COLS = NBLK * 4 * 512     # XP free cols (4 timesteps per block)

# ---- DVE slice geometry ----
P = 127
B_DVE = B_CORE - B_PE        # 303_392
N_CHUNKS = 3
G = 797                      # columns per chunk
G_TOTAL = G * N_CHUNKS       # 2391 ; 127*2391 = 303_657 >= B_DVE
B_PAD = P * G_TOTAL
CHUNK_COLS = 8 * G           # u:4G then v:4G
COLS = N_CHUNKS * CHUNK_COLS

WI = [[0.3519, -0.6514], [0.3238, 0.5568]]
BI = [0.2198, 0.4712]
WH = [[0.4279, 0.6832], [-0.4114, 0.5715]]
BH = [-0.409, -0.1299]
CW = [-0.2732, -0.1587]
CB = 0.5806

_cached_nc = None


def _const_pack() -> np.ndarray:
    """[128, 321]: Wx(128) | Wh(128) | Wc(64) | bias(1), block-diagonal."""
    ct = np.zeros((128, 321), np.float32)
    for g in range(64):
        for k in range(2):
            for h in range(2):
                ct[2 * g + k, 2 * g + h] = WI[h][k]
                ct[2 * g + k, 128 + 2 * g + h] = WH[h][k]
            ct[2 * g + k, 256 + g] = CW[k]
    for h in range(2):
        ct[np.arange(64) * 2 + h, 320] = BI[h] + BH[h]
    return ct


def build_program():
    import concourse.bass as bass  # noqa: F401
    import concourse.tile as tile
    from concourse import bacc, mybir

    f32 = mybir.dt.float32
    mult = mybir.AluOpType.mult
    add = mybir.AluOpType.add
    Tanh = mybir.ActivationFunctionType.Tanh
    Copy = mybir.ActivationFunctionType.Copy

    r = [WI[0][0] / WI[0][1], WI[1][0] / WI[1][1]]
    a = [
        [WH[0][0] / WI[0][1], WH[0][1] / WI[0][1]],
        [WH[1][0] / WI[1][1], WH[1][1] / WI[1][1]],
    ]
    act_scale = [WI[0][1], WI[1][1]]
    act_bias = [BI[0] + BH[0], BI[1] + BH[1]]
    ccoef = CW[0] / CW[1]

    nc = bacc.Bacc(None, target_bir_lowering=False)

    for val in act_bias:
        t = nc.alloc_sbuf_tensor(f"const-bias-{val}", [128, 1], f32)
        nc.gpsimd.memset(t.ap(), val)
        nc.const_aps.aps[(f32, val)] = t.ap()
    nc.all_engine_barrier()

    x_d = nc.dram_tensor("X", [128, COLS], f32, kind="ExternalInput")
    xp_d = nc.dram_tensor("XP", [128, PE_COLS], f32, kind="ExternalInput")
    ct_d = nc.dram_tensor("CT", [128, 321], f32, kind="ExternalInput")
    o_d = nc.dram_tensor("out", [128, G_TOTAL], f32, kind="ExternalOutput")
    op_d = nc.dram_tensor("out_pe", [64, NBLK * 512], f32, kind="ExternalOutput")

    with tile.TileContext(nc) as tc:
        with (
            tc.tile_pool(name="io", bufs=2) as io_pool,
            tc.tile_pool(name="work", bufs=2) as work,
            tc.tile_pool(name="persist", bufs=1) as persist,
            tc.tile_pool(name="ps", bufs=1, space="PSUM") as ps,
        ):
            ct = persist.tile([128, 321], f32, tag="ct")
            nc.sync.dma_start(out=ct, in_=ct_d[:])
            wx = ct[:, 0:128]
            wh = ct[:, 128:256]
            wc = ct[:, 256:320]
            bias_ap = ct[:, 320:321]

            out_acc = persist.tile([128, G_TOTAL], f32, tag="out_acc")
            op_acc = persist.tile([64, NBLK * 512], f32, tag="op_acc")

            def pe_pair(pair):
                """Two 512-col blocks through the 4-step recurrence on PE."""
                jb = (2 * pair, 2 * pair + 1)
                pxs = []
                for jj, b in enumerate(jb):
                    px = io_pool.tile([128, 4 * 512], f32, tag=f"px{jj}")
                    nc.scalar.dma_start(
                        out=px, in_=xp_d[:, b * 2048 : (b + 1) * 2048]
                    )
                    pxs.append(px)
                phs = {}
                for t in range(4):
                    zps = []
                    for jj in range(2):
                        zp = ps.tile([128, 512], f32, tag=f"pz{jj}{t % 2}")
                        nc.tensor.matmul(
                            zp, wx, pxs[jj][:, t * 512 : (t + 1) * 512],
                            start=True, stop=(t == 0),
                        )
                        zps.append(zp)
                    if t > 0:
                        for jj in range(2):
                            nc.tensor.matmul(
                                zps[jj], wh, phs[(jj, t - 1)],
                                start=False, stop=True,
                            )
                    for jj in range(2):
                        ph = work.tile([128, 512], f32, tag=f"ph{jj}{t % 2}", bufs=1)
                        nc.scalar.activation(
                            out=ph, in_=zps[jj], func=Tanh,
                            bias=bias_ap, scale=1.0,
                        )
                        phs[(jj, t)] = ph
                qs = []
                for jj in range(2):
                    zq = ps.tile([64, 512], f32, tag=f"pq{jj}")
                    nc.tensor.matmul(zq, wc, phs[(jj, 3)], start=True, stop=True)
                    qs.append(zq)
                for jj, b in enumerate(jb):
                    nc.scalar.activation(
                        out=op_acc[:, b * 512 : (b + 1) * 512],
                        in_=qs[jj], func=Copy, bias=CB, scale=1.0,
                    )
                nc.gpsimd.dma_start(
                    out=op_d[:][:, jb[0] * 512 : (jb[1] + 1) * 512],
                    in_=op_acc[:, jb[0] * 512 : (jb[1] + 1) * 512],
                )

            def dve_chunk(j):
                xt = io_pool.tile([128, CHUNK_COLS], f32, tag="xt")
                nc.sync.dma_start(
                    out=xt, in_=x_d[:][:, j * CHUNK_COLS : (j + 1) * CHUNK_COLS]
                )
                u_all = xt[:, 0 : 4 * G]
                v_all = xt[:, 4 * G : 8 * G]
                s = []
                for h in range(2):
                    sh = work.tile([128, 4 * G], f32, tag=f"s{h}", bufs=1)
                    nc.vector.scalar_tensor_tensor(
                        out=sh, in0=u_all, scalar=r[h], in1=v_all,
                        op0=mult, op1=add,
                    )
                    s.append(sh)
                hcur = []
                for h in range(2):
                    ht = work.tile([128, G], f32, tag=f"h{h}0", bufs=1)
                    nc.scalar.activation(
                        out=ht, in_=s[h][:, 0:G], func=Tanh,
                        bias=act_bias[h], scale=act_scale[h],
                    )
                    hcur.append(ht)
                for t in range(1, 4):
                    hprev = hcur
                    hcur = []
                    for h in range(2):
                        tmp = work.tile([128, G], f32, tag=f"tmp{h}")
                        nc.vector.scalar_tensor_tensor(
                            out=tmp, in0=hprev[0], scalar=a[h][0],
                            in1=s[h][:, t * G : (t + 1) * G],
                            op0=mult, op1=add,
                        )
                        z = work.tile([128, G], f32, tag=f"z{h}")
                        nc.vector.scalar_tensor_tensor(
                            out=z, in0=hprev[1], scalar=a[h][1], in1=tmp,
                            op0=mult, op1=add,
                        )
                        ht = work.tile([128, G], f32, tag=f"h{h}{t % 2}", bufs=1)
                        nc.scalar.activation(
                            out=ht, in_=z, func=Tanh,
                            bias=act_bias[h], scale=act_scale[h],
                        )
                        hcur.append(ht)
                q = work.tile([128, G], f32, tag="q")
                nc.vector.scalar_tensor_tensor(
                    out=q, in0=hcur[0], scalar=ccoef, in1=hcur[1],
                    op0=mult, op1=add,
                )
                nc.scalar.activation(
                    out=out_acc[:, j * G : (j + 1) * G],
                    in_=q, func=Copy, bias=CB, scale=CW[1],
                )
                nc.gpsimd.dma_start(
                    out=o_d[:][:, j * G : (j + 1) * G],
                    in_=out_acc[:, j * G : (j + 1) * G],
                )

            # interleave the two pipelines so neither starves on ACT
            for i in range(max(NBLK // 2, N_CHUNKS)):
                if i < NBLK // 2:
                    pe_pair(i)
                if i < N_CHUNKS:
                    dve_chunk(i)

    nc.compile()
    return nc


def _get_nc():
    global _cached_nc
    if _cached_nc is None:
        _cached_nc = build_program()
    return _cached_nc


_CT = None


def _pack_core(xc: np.ndarray) -> dict:
    """[B_CORE, 4, 2] -> {"X": [128, COLS], "XP": [128, PE_COLS], "CT": ...}."""
    global _CT
    if _CT is None:
        _CT = _const_pack()
    xd = xc[:B_DVE]
    pad = np.zeros((B_PAD, 8), np.float32)
    pad[:B_DVE] = xd.reshape(B_DVE, 8)
    v = pad.reshape(P, N_CHUNKS, G, 4, 2).transpose(0, 1, 4, 3, 2)
    bx = np.zeros((128, COLS), np.float32)
    bx[:P] = v.reshape(P, COLS)

    xpe = xc[B_DVE:].reshape(NBLK, 64, 512, 4, 2)  # blk, g, c, t, k
    bp = np.ascontiguousarray(
        xpe.transpose(1, 4, 0, 3, 2).reshape(128, PE_COLS)
    )
    return {"X": bx, "XP": bp, "CT": _CT}


def _unpack_core(res_i: dict) -> np.ndarray:
    od = res_i["out"][:P].reshape(-1)[:B_DVE]
    op = res_i["out_pe"].reshape(64, NBLK, 512).transpose(1, 0, 2).reshape(-1)
    return np.concatenate([od, op])


def run_sharded(X: np.ndarray, trace: bool = False):
    """Run the SPMD kernel on 8 cores. Returns (out_full, BassKernelResults)."""
    from concourse import bass_utils

    nc = _get_nc()
    X = np.ascontiguousarray(np.asarray(X, dtype=np.float32))
    assert X.shape == (B_TOTAL, 4, 2), X.shape
    in_maps = [
        _pack_core(X[i * B_CORE : (i + 1) * B_CORE]) for i in range(N_CORES)
    ]
    res = bass_utils.run_bass_kernel_spmd(
        nc, in_maps, core_ids=list(range(N_CORES)), trace=trace
    )
    out = np.concatenate(
        [_unpack_core(res.results[i]) for i in range(N_CORES)]
    ).reshape(B_TOTAL, 1)
    return out, res


def kernel(**inputs: np.ndarray) -> np.ndarray:
    out, _ = run_sharded(inputs["X"])
    return out.astype(np.float32)
